# revision 28
# baseline (speedup 1.0000x reference)
"""Multi-head attention (B=8, N=1024, D=512, H=8) on 8 TRN2 NeuronCores.

Sharding: pure batch-parallel — core i computes batch i end-to-end, no
collectives. Host-side prep per batch: gather valid keys (mask) into a
contiguous buffer padded to NKV=640, pre-transpose x, and build small
per-partition bias/validity tables. Device pipeline (all matmuls fp32r):
  q^T/k^T projections -> scores s^T[k,q] per head -> exp on ACT with the
  key-padding mask folded into the per-partition activation bias ->
  attn@v with an augmented ones-column producing the softmax denominator
  in row 64 -> reciprocal + PE-broadcast -> normalize -> out-projection.
"""

import sys

import numpy as np

sys.path.insert(0, "/opt/trn_rl_repo")

B, N, D, H = 8, 1024, 512, 8
HD = D // H            # 64
SCALE = HD ** -0.5     # 0.125
NKV = 640              # padded valid-key count (5 chunks of 128)
KC = NKV // 128        # 5
DC = D // 128          # 4
PAD_BIAS = -30000.0    # exp(PAD_BIAS*1 + s*SCALE) == 0.0 exactly in fp32

_prog_cache = {}


def _build_program():
    import concourse.bacc as bacc
    import concourse.tile as tile
    from concourse import mybir

    dt = mybir.dt
    f32 = dt.float32
    f32r = dt.float32r
    AF = mybir.ActivationFunctionType

    def r(ap):
        return ap.bitcast(f32r)

    nc = bacc.Bacc("TRN2", target_bir_lowering=False, debug=False)

    xT_d = nc.dram_tensor("xT", [D, N], f32, kind="ExternalInput").ap()
    xkT_d = nc.dram_tensor("xkT", [D, NKV], f32, kind="ExternalInput").ap()
    wq_d = nc.dram_tensor("wq", [D, D], f32, kind="ExternalInput").ap()
    wk_d = nc.dram_tensor("wk", [D, D], f32, kind="ExternalInput").ap()
    wv_d = nc.dram_tensor("wv", [D, D], f32, kind="ExternalInput").ap()
    wo_d = nc.dram_tensor("wo", [D, D], f32, kind="ExternalInput").ap()
    bq_d = nc.dram_tensor("bq", [D, 1], f32, kind="ExternalInput").ap()
    bk_d = nc.dram_tensor("bk", [D, 1], f32, kind="ExternalInput").ap()
    bvb_d = nc.dram_tensor("bvb", [128, D], f32, kind="ExternalInput").ap()
    bob_d = nc.dram_tensor("bob", [128, D], f32, kind="ExternalInput").ap()
    expb_d = nc.dram_tensor("expb", [128, KC], f32, kind="ExternalInput").ap()
    ones1_d = nc.dram_tensor("ones1", [1, HD], f32, kind="ExternalInput").ap()
    onesv_d = nc.dram_tensor("onesv", [128, H, 1], f32, kind="ExternalInput").ap()
    y_d = nc.dram_tensor("y", [N, D], f32, kind="ExternalOutput").ap()

    with tile.TileContext(nc) as tc, \
         nc.allow_low_precision(reason="float32r == fp32 rounded for PE fast path"):
        with tc.tile_pool(name="const", bufs=1) as cpool:
            def load_w(name, dram):
                tiles = []
                for c in range(DC):
                    t = cpool.tile([128, D], f32, name=f"{name}{c}")
                    nc.sync.dma_start(r(t[:]), r(dram[128 * c:128 * (c + 1), :]))
                    tiles.append(t)
                return tiles

            wq_t = load_w("wq_t", wq_d)
            wk_t = load_w("wk_t", wk_d)
            wv_t = load_w("wv_t", wv_d)
            wo_t = load_w("wo_t", wo_d)

            xT_t = []
            for c in range(DC):
                t = cpool.tile([128, N], f32, name=f"xT_t{c}")
                nc.sync.dma_start(r(t[:]), r(xT_d[128 * c:128 * (c + 1), :]))
                xT_t.append(t)
            xkT_t = []
            for c in range(DC):
                t = cpool.tile([128, NKV], f32, name=f"xkT_t{c}")
                nc.sync.dma_start(r(t[:]), r(xkT_d[128 * c:128 * (c + 1), :]))
                xkT_t.append(t)

            bq_t = cpool.tile([128, DC], f32, name="bq_t")
            bk_t = cpool.tile([128, DC], f32, name="bk_t")
            for c in range(DC):
                nc.sync.dma_start(bq_t[:, c:c + 1], bq_d[128 * c:128 * (c + 1), :])
                nc.sync.dma_start(bk_t[:, c:c + 1], bk_d[128 * c:128 * (c + 1), :])
            bvb_t = cpool.tile([128, H, HD], f32, name="bvb_t")
            nc.sync.dma_start(bvb_t[:], bvb_d[:, :])
            bob_t = cpool.tile([128, D], f32, name="bob_t")
            nc.sync.dma_start(bob_t[:], bob_d[:, :])
            expb_t = cpool.tile([128, KC], f32, name="expb_t")
            nc.sync.dma_start(expb_t[:], expb_d[:, :])
            ones_t = cpool.tile([1, HD], f32, name="ones_t")
            nc.sync.dma_start(r(ones_t[:]), r(ones1_d[:]))

            qT_t = [cpool.tile([128, N], f32, name=f"qT_t{c}") for c in range(DC)]
            kT_t = [cpool.tile([128, NKV], f32, name=f"kT_t{c}") for c in range(DC)]
            vaug_t = [cpool.tile([128, H, HD + 1], f32, name=f"vaug_t{c}")
                      for c in range(KC)]
            aoT_t = [cpool.tile([128, N], f32, name=f"aoT_t{c}") for c in range(DC)]

            # ---- Phase 1: q/k/v projections ----
            with tc.tile_pool(name="qpp", bufs=2, space="PSUM") as qpp:
                for dp in range(DC):
                    ps = qpp.tile([128, N], f32, name="qps")
                    for dc in range(DC):
                        lhs = r(wq_t[dc][:, 128 * dp:128 * (dp + 1)])
                        for hf in range(2):
                            nc.tensor.matmul(
                                ps[:, 512 * hf:512 * (hf + 1)],
                                lhs,
                                r(xT_t[dc][:, 512 * hf:512 * (hf + 1)]),
                                start=(dc == 0), stop=(dc == DC - 1),
                            )
                    nc.vector.tensor_scalar_add(r(qT_t[dp][:]), ps[:], bq_t[:, dp:dp + 1])

            with tc.tile_pool(name="kpp", bufs=2, space="PSUM") as kpp:
                for dp in range(DC):
                    ps = kpp.tile([128, NKV], f32, name="kps")
                    for dc in range(DC):
                        lhs = r(wk_t[dc][:, 128 * dp:128 * (dp + 1)])
                        nc.tensor.matmul(
                            ps[:, 0:512], lhs, r(xkT_t[dc][:, 0:512]),
                            start=(dc == 0), stop=(dc == DC - 1),
                        )
                        nc.tensor.matmul(
                            ps[:, 512:NKV], lhs, r(xkT_t[dc][:, 512:NKV]),
                            start=(dc == 0), stop=(dc == DC - 1),
                        )
                    nc.vector.tensor_scalar_add(r(kT_t[dp][:]), ps[:], bk_t[:, dp:dp + 1])

            with tc.tile_pool(name="vpp", bufs=2, space="PSUM") as vpp:
                for c in range(KC):
                    ps = vpp.tile([128, H, HD], f32, name="vps")
                    for dc in range(DC):
                        nc.tensor.matmul(
                            ps[:], r(xkT_t[dc][:, 128 * c:128 * (c + 1)]),
                            r(wv_t[dc][:]),
                            start=(dc == 0), stop=(dc == DC - 1),
                        )
                    nc.vector.tensor_add(
                        r(vaug_t[c][:, :, 0:HD]), ps[:], bvb_t[:])
                    nc.sync.dma_start(r(vaug_t[c][:, :, HD:HD + 1]), r(onesv_d[:]))

            # ---- Phase 2: attention per head ----
            with tc.tile_pool(name="scp", bufs=2, space="PSUM") as scp, \
                 tc.tile_pool(name="oap", bufs=3, space="PSUM") as oap, \
                 tc.tile_pool(name="rbp", bufs=1, space="PSUM") as rbp, \
                 tc.tile_pool(name="pp", bufs=3) as pp, \
                 tc.tile_pool(name="rcp", bufs=2) as rcp:
                for h in range(H):
                    dp, row = h // 2, HD * (h % 2)
                    oa = [oap.tile([HD + 1, 512], f32, name="oa")
                          for hf in range(2)]
                    for c in range(KC):
                        sc = scp.tile([128, N], f32, name="sc")
                        for hf in range(2):
                            nc.tensor.matmul(
                                sc[:, 512 * hf:512 * (hf + 1)],
                                r(kT_t[dp][row:row + HD, 128 * c:128 * (c + 1)]),
                                r(qT_t[dp][row:row + HD, 512 * hf:512 * (hf + 1)]),
                                start=True, stop=True,
                            )
                        p = pp.tile([128, N], f32, name="p")
                        nc.scalar.activation(
                            r(p[:]), sc[:], AF.Exp,
                            bias=expb_t[:, c:c + 1], scale=SCALE,
                        )
                        for hf in range(2):
                            nc.tensor.matmul(
                                oa[hf][:],
                                r(vaug_t[c][:, h, :]),
                                r(p[:, 512 * hf:512 * (hf + 1)]),
                                start=(c == 0), stop=(c == KC - 1),
                            )
                    for hf in range(2):
                        rc = rcp.tile([1, 512], f32, name="rc")
                        nc.vector.reciprocal(r(rc[:]), oa[hf][HD:HD + 1, :])
                        rb = rbp.tile([HD, 512], f32, name="rb")
                        nc.tensor.matmul(rb[:], r(ones_t[:]), r(rc[:]),
                                         start=True, stop=True)
                        rbs = rcp.tile([HD, 512], f32, name="rbs")
                        nc.vector.tensor_scalar_add(rbs[:], rb[:], 0.0)
                        nc.vector.tensor_mul(
                            r(aoT_t[dp][row:row + HD, 512 * hf:512 * (hf + 1)]),
                            oa[hf][0:HD, :], rbs[:])

            # ---- Phase 3: output projection ----
            with tc.tile_pool(name="ypp", bufs=2, space="PSUM") as ypp, \
                 tc.tile_pool(name="ysp", bufs=2) as ysp:
                for ic in range(N // 128):
                    yps = ypp.tile([128, D], f32, name="yps")
                    for dp in range(DC):
                        nc.tensor.matmul(
                            yps[:], r(aoT_t[dp][:, 128 * ic:128 * (ic + 1)]),
                            r(wo_t[dp][:]),
                            start=(dp == 0), stop=(dp == DC - 1),
                        )
                    ysb = ysp.tile([128, D], f32, name="ysb")
                    nc.vector.tensor_add(ysb[:], yps[:], bob_t[:])
                    nc.sync.dma_start(y_d[128 * ic:128 * (ic + 1), :], ysb[:])

    return nc


def _get_program():
    if "nc" not in _prog_cache:
        nc = _build_program()
        if not nc.is_finalized():
            nc.finalize()
        _prog_cache["nc"] = nc
    return _prog_cache["nc"]


def _round_fp32r(a):
    # fp32r = fp32 with the mantissa rounded (RNE) to 11 bits (low 12 bits 0)
    bits = np.ascontiguousarray(a, np.float32).view(np.uint32)
    low = bits & np.uint32(0xFFF)
    base = bits & np.uint32(0xFFFFF000)
    lsb = (base >> np.uint32(12)) & np.uint32(1)
    rnd = (low > 0x800) | ((low == 0x800) & (lsb == 1))
    return (base + (rnd.astype(np.uint32) << np.uint32(12))).view(np.float32)


def _prep_core(b, x, mask, wq, bq, wk, bk, wv, bv, wo, bo):
    xb = np.ascontiguousarray(x[b], dtype=np.float32)       # [N, D]
    idx = np.nonzero(mask[b])[0]
    nv = int(idx.size)
    assert 1 <= nv <= NKV, f"batch {b}: {nv} valid keys, NKV={NKV}"
    xk = np.zeros((NKV, D), np.float32)
    xk[:nv] = xb[idx]
    pos = np.arange(128)[:, None] + 128 * np.arange(KC)[None, :]
    expb = np.where(pos < nv, 0.0, PAD_BIAS).astype(np.float32)
    f = np.float32
    return {
        "xT": _round_fp32r(xb.T),
        "xkT": _round_fp32r(xk.T),
        "wq": _round_fp32r(wq), "wk": _round_fp32r(wk),
        "wv": _round_fp32r(wv), "wo": _round_fp32r(wo),
        "bq": np.ascontiguousarray(bq, f).reshape(D, 1),
        "bk": np.ascontiguousarray(bk, f).reshape(D, 1),
        "bvb": np.ascontiguousarray(np.broadcast_to(bv.astype(f), (128, D))),
        "bob": np.ascontiguousarray(np.broadcast_to(bo.astype(f), (128, D))),
        "expb": expb,
        "ones1": np.ones((1, HD), f),
        "onesv": np.ones((128, H, 1), f),
    }


def _run(inputs):
    import os

    os.environ["BASS_NEVER_TRACE"] = "1"
    from concourse.bass_utils import run_bass_kernel_spmd

    nc = _get_program()
    in_maps = [_prep_core(b, **inputs) for b in range(B)]
    res = run_bass_kernel_spmd(nc, in_maps, core_ids=list(range(B)), trace=False)
    out = np.stack([res.results[b]["y"] for b in range(B)], axis=0)
    return out.astype(np.float32), res


def kernel(**inputs) -> np.ndarray:
    out, _ = _run(inputs)
    return out


# revision 33
# speedup vs baseline: 1.2336x; 1.2336x over previous
"""Multi-head attention (B=8, N=1024, D=512, H=8) on 8 TRN2 NeuronCores.

Sharding: pure batch-parallel — core i computes batch i end-to-end, no
collectives. Host-side prep per batch: gather valid keys (mask) into a
contiguous buffer padded to NKV=640, pre-transpose x, and build small
per-partition bias/validity tables. Device pipeline (all matmuls fp32r):
  k^T/q^T/v projections -> scores s^T[k,q] per head -> exp on ACT with the
  key-padding mask folded into the per-partition activation bias ->
  attn@v with an augmented ones-column producing the softmax denominator
  in row 64 -> fast reciprocal + Pool partition-broadcast -> normalize ->
  out-projection.

Math shortcuts: bk is dropped (constant-in-key terms cancel in softmax);
bv is folded into the output bias on the host (bob' = bo + bv @ wo since
normalized attention rows sum to 1).
"""

import sys

import numpy as np

sys.path.insert(0, "/opt/trn_rl_repo")

B, N, D, H = 8, 1024, 512, 8
HD = D // H            # 64
SCALE = HD ** -0.5     # 0.125
NKV = 640              # padded valid-key count (5 chunks of 128)
KC = NKV // 128        # 5
DC = D // 128          # 4
PAD_BIAS = -30000.0    # exp(PAD_BIAS + s*SCALE) == 0.0 exactly in fp32

_prog_cache = {}


def _build_program():
    import concourse.bacc as bacc
    import concourse.tile as tile
    from concourse import mybir

    dt = mybir.dt
    f32 = dt.float32
    f32r = dt.float32r
    AF = mybir.ActivationFunctionType

    def r(ap):
        return ap.bitcast(f32r)

    nc = bacc.Bacc("TRN2", target_bir_lowering=False, debug=False)

    xT_d = nc.dram_tensor("xT", [D, N], f32, kind="ExternalInput").ap()
    xkT_d = nc.dram_tensor("xkT", [D, NKV], f32, kind="ExternalInput").ap()
    wq_d = nc.dram_tensor("wq", [D, D], f32, kind="ExternalInput").ap()
    wk_d = nc.dram_tensor("wk", [D, D], f32, kind="ExternalInput").ap()
    wv_d = nc.dram_tensor("wv", [D, D], f32, kind="ExternalInput").ap()
    wo_d = nc.dram_tensor("wo", [D, D], f32, kind="ExternalInput").ap()
    bq_d = nc.dram_tensor("bq", [D, 1], f32, kind="ExternalInput").ap()
    bob_d = nc.dram_tensor("bob", [128, D], f32, kind="ExternalInput").ap()
    expb_d = nc.dram_tensor("expb", [128, KC], f32, kind="ExternalInput").ap()
    onesv_d = nc.dram_tensor("onesv", [128, H, 1], f32, kind="ExternalInput").ap()
    y_d = nc.dram_tensor("y", [N, D], f32, kind="ExternalOutput").ap()

    with tile.TileContext(nc) as tc, \
         nc.allow_low_precision(reason="float32r == fp32 rounded for PE fast path"):
        with tc.tile_pool(name="const", bufs=1) as cpool:
            # Persistent result tiles (vaug gets its ones column via DMA below)
            qT_t = [cpool.tile([128, N], f32, name=f"qT_t{c}") for c in range(DC)]
            kT_t = [cpool.tile([128, NKV], f32, name=f"kT_t{c}") for c in range(DC)]
            vaug_t = [cpool.tile([128, H, HD + 1], f32, name=f"vaug_t{c}")
                      for c in range(KC)]
            aoT_t = [cpool.tile([128, N], f32, name=f"aoT_t{c}") for c in range(DC)]

            # --- DMA issue order = priority order (queues drain round-robin).
            # Small tables first, then k deps, q deps, v, and wo/bob last.
            bq_t = cpool.tile([128, DC], f32, name="bq_t")
            for c in range(DC):
                nc.sync.dma_start(bq_t[:, c:c + 1], bq_d[128 * c:128 * (c + 1), :])
            expb_t = cpool.tile([128, KC], f32, name="expb_t")
            nc.sync.dma_start(expb_t[:], expb_d[:, :])
            for c in range(KC):
                nc.sync.dma_start(r(vaug_t[c][:, :, HD:HD + 1]), r(onesv_d[:]))

            def load_w(name, dram):
                tiles = []
                for c in range(DC):
                    t = cpool.tile([128, D], f32, name=f"{name}{c}")
                    nc.sync.dma_start(r(t[:]), r(dram[128 * c:128 * (c + 1), :]))
                    tiles.append(t)
                return tiles

            wk_t = load_w("wk_t", wk_d)
            xkT_t = []
            for c in range(DC):
                t = cpool.tile([128, NKV], f32, name=f"xkT_t{c}")
                nc.sync.dma_start(r(t[:]), r(xkT_d[128 * c:128 * (c + 1), :]))
                xkT_t.append(t)
            wq_t = load_w("wq_t", wq_d)
            xT_t = []
            for c in range(DC):
                t = cpool.tile([128, N], f32, name=f"xT_t{c}")
                nc.sync.dma_start(r(t[:]), r(xT_d[128 * c:128 * (c + 1), :]))
                xT_t.append(t)
            wv_t = load_w("wv_t", wv_d)
            wo_t = load_w("wo_t", wo_d)
            bob_t = cpool.tile([128, D], f32, name="bob_t")
            nc.sync.dma_start(bob_t[:], bob_d[:, :])

            # ---- Phase 1a: k projection (no bias: cancels in softmax) ----
            with tc.tile_pool(name="kpp", bufs=2, space="PSUM") as kpp:
                for dp in range(DC):
                    ps = kpp.tile([128, NKV], f32, name="kps")
                    for dc in range(DC):
                        lhs = r(wk_t[dc][:, 128 * dp:128 * (dp + 1)])
                        nc.tensor.matmul(
                            ps[:, 0:512], lhs, r(xkT_t[dc][:, 0:512]),
                            start=(dc == 0), stop=(dc == DC - 1),
                        )
                        nc.tensor.matmul(
                            ps[:, 512:NKV], lhs, r(xkT_t[dc][:, 512:NKV]),
                            start=(dc == 0), stop=(dc == DC - 1),
                        )
                    nc.vector.tensor_scalar_add(r(kT_t[dp][:]), ps[:], 0.0)

            # ---- Phase 1b: q projection ----
            with tc.tile_pool(name="qpp", bufs=2, space="PSUM") as qpp:
                for dp in range(DC):
                    ps = qpp.tile([128, N], f32, name="qps")
                    for dc in range(DC):
                        lhs = r(wq_t[dc][:, 128 * dp:128 * (dp + 1)])
                        for hf in range(2):
                            nc.tensor.matmul(
                                ps[:, 512 * hf:512 * (hf + 1)],
                                lhs,
                                r(xT_t[dc][:, 512 * hf:512 * (hf + 1)]),
                                start=(dc == 0), stop=(dc == DC - 1),
                            )
                    nc.vector.tensor_scalar_add(r(qT_t[dp][:]), ps[:], bq_t[:, dp:dp + 1])

            # ---- Phase 1c: v projection (no bias: folded into bob') ----
            with tc.tile_pool(name="vpp", bufs=2, space="PSUM") as vpp:
                for c in range(KC):
                    ps = vpp.tile([128, H, HD], f32, name="vps")
                    for dc in range(DC):
                        nc.tensor.matmul(
                            ps[:], r(xkT_t[dc][:, 128 * c:128 * (c + 1)]),
                            r(wv_t[dc][:]),
                            start=(dc == 0), stop=(dc == DC - 1),
                        )
                    nc.vector.tensor_scalar_add(r(vaug_t[c][:, :, 0:HD]), ps[:], 0.0)

            # ---- Phase 2: attention per head (score/exp/AV pipelined) ----
            with tc.tile_pool(name="scp", bufs=2, space="PSUM") as scp, \
                 tc.tile_pool(name="oap", bufs=4, space="PSUM") as oap, \
                 tc.tile_pool(name="pp", bufs=3) as pp, \
                 tc.tile_pool(name="rcp", bufs=6) as rcp:
                SKEW = 2

                for h in range(H):
                    dp, row = h // 2, HD * (h % 2)
                    oa = [oap.tile([HD + 1, 512], f32, name="oa")
                          for hf in range(2)]
                    p_t = []

                    def av(cav):
                        for hf in range(2):
                            nc.tensor.matmul(
                                oa[hf][:],
                                r(vaug_t[cav][:, h, :]),
                                r(p_t[cav][:, 512 * hf:512 * (hf + 1)]),
                                start=(cav == 0), stop=(cav == KC - 1),
                            )

                    for c in range(KC):
                        sc = scp.tile([128, N], f32, name="sc")
                        for hf in range(2):
                            nc.tensor.matmul(
                                sc[:, 512 * hf:512 * (hf + 1)],
                                r(kT_t[dp][row:row + HD, 128 * c:128 * (c + 1)]),
                                r(qT_t[dp][row:row + HD, 512 * hf:512 * (hf + 1)]),
                                start=True, stop=True,
                            )
                        p = pp.tile([128, N], f32, name="p")
                        nc.scalar.activation(
                            r(p[:]), sc[:], AF.Exp,
                            bias=expb_t[:, c:c + 1], scale=SCALE,
                        )
                        p_t.append(p)
                        if c >= SKEW:
                            av(c - SKEW)
                    for cav in range(KC - SKEW, KC):
                        av(cav)

                    for hf in range(2):
                        # custom DVE ops read garbage from PSUM on HW: stage
                        # the denominator row through SBUF via ACT copy
                        db = rcp.tile([1, 512], f32, name="db")
                        nc.scalar.copy(db[:], oa[hf][HD:HD + 1, :])
                        rc = rcp.tile([1, 512], f32, name="rc")
                        nc.vector.reciprocal_approx_fast(rc[:], db[:])
                        rbs = rcp.tile([HD, 512], f32, name="rbs")
                        nc.gpsimd.partition_broadcast(rbs[:], rc[:])
                        nc.vector.tensor_mul(
                            r(aoT_t[dp][row:row + HD, 512 * hf:512 * (hf + 1)]),
                            oa[hf][0:HD, :], rbs[:])

            # ---- Phase 3: output projection ----
            with tc.tile_pool(name="ypp", bufs=2, space="PSUM") as ypp, \
                 tc.tile_pool(name="ysp", bufs=2) as ysp:
                for ic in range(N // 128):
                    yps = ypp.tile([128, D], f32, name="yps")
                    for dp in range(DC):
                        nc.tensor.matmul(
                            yps[:], r(aoT_t[dp][:, 128 * ic:128 * (ic + 1)]),
                            r(wo_t[dp][:]),
                            start=(dp == 0), stop=(dp == DC - 1),
                        )
                    ysb = ysp.tile([128, D], f32, name="ysb")
                    nc.vector.tensor_add(ysb[:], yps[:], bob_t[:])
                    nc.sync.dma_start(y_d[128 * ic:128 * (ic + 1), :], ysb[:])

    return nc


def _get_program():
    if "nc" not in _prog_cache:
        nc = _build_program()
        if not nc.is_finalized():
            nc.finalize()
        _prog_cache["nc"] = nc
    return _prog_cache["nc"]


def _round_fp32r(a):
    # fp32r = fp32 with the mantissa rounded (RNE) to 11 bits (low 12 bits 0)
    bits = np.ascontiguousarray(a, np.float32).view(np.uint32)
    low = bits & np.uint32(0xFFF)
    base = bits & np.uint32(0xFFFFF000)
    lsb = (base >> np.uint32(12)) & np.uint32(1)
    rnd = (low > 0x800) | ((low == 0x800) & (lsb == 1))
    return (base + (rnd.astype(np.uint32) << np.uint32(12))).view(np.float32)


def _prep_core(b, x, mask, wq, bq, wk, bk, wv, bv, wo, bo):
    xb = np.ascontiguousarray(x[b], dtype=np.float32)       # [N, D]
    idx = np.nonzero(mask[b])[0]
    nv = int(idx.size)
    assert 1 <= nv <= NKV, f"batch {b}: {nv} valid keys, NKV={NKV}"
    xk = np.zeros((NKV, D), np.float32)
    xk[:nv] = xb[idx]
    pos = np.arange(128)[:, None] + 128 * np.arange(KC)[None, :]
    expb = np.where(pos < nv, 0.0, PAD_BIAS).astype(np.float32)
    f = np.float32
    bob = (bo.astype(f) + bv.astype(f) @ wo.astype(f)).reshape(D)
    return {
        "xT": _round_fp32r(xb.T),
        "xkT": _round_fp32r(xk.T),
        "wq": _round_fp32r(wq), "wk": _round_fp32r(wk),
        "wv": _round_fp32r(wv), "wo": _round_fp32r(wo),
        "bq": np.ascontiguousarray(bq, f).reshape(D, 1),
        "bob": np.ascontiguousarray(np.broadcast_to(bob, (128, D))),
        "expb": expb,
        "onesv": np.ones((128, H, 1), f),
    }


def _run(inputs):
    import os

    os.environ["BASS_NEVER_TRACE"] = "1"
    from concourse.bass_utils import run_bass_kernel_spmd

    nc = _get_program()
    in_maps = [_prep_core(b, **inputs) for b in range(B)]
    res = run_bass_kernel_spmd(nc, in_maps, core_ids=list(range(B)), trace=False)
    out = np.stack([res.results[b]["y"] for b in range(B)], axis=0)
    return out.astype(np.float32), res


def kernel(**inputs) -> np.ndarray:
    out, _ = _run(inputs)
    return out
